# revision 15
# baseline (speedup 1.0000x reference)
"""CoAttentionFusion Trainium2 kernel (8 NeuronCores, SPMD, no collectives).

Sharding: core c = (batch b = c//2, query-half h = c%2). Each core computes
the full module for its 1024 query rows of batch b; K/V projections over the
full T=2048 are recomputed by both cores of a batch pair (21% redundant
compute, zero communication).

On-chip strategy:
  - activations feature-major (x^T: [d, tokens]) so every linear layer is
    lhsT = W (as stored, [din, dout]), rhs = x^T -> y^T, no transposes.
  - attention computed with transposed scores S^T[k, q] = K^T_h . Q_h^T so the
    exp'd probabilities P^T are directly the moving operand of P@V.
  - V produced token-major with a ones-column appended; the P@V accumulation
    then yields O'^T = [rawO^T ; softmax-denominator] in one group.
  - normalization of O via DVE reciprocal + GPSIMD partition_broadcast.
  - LayerNorms run token-major (per-partition stats) on 128-token chunks,
    entering/leaving via PE transposes.
  - SBUF is tight: x / K / V / O are streamed through DRAM scratch in 512-token
    blocks; K/V projections for attention-2 are emitted interleaved with
    attention-1 (and O-proj/LN of stream t with attention-2) to keep PE busy
    while the ACT engine grinds through exp().
All matmuls bf16 with fp32 PSUM accumulation; softmax/LN math in fp32.
"""

import numpy as np

P = 128
D = 1024
T = 2048
TQ = 1024
NH = 16
HD = 64
DT = D // P          # 8 feature tiles
KT = T // P          # 16 key-token tiles
QC = TQ // P         # 8 query-token chunks
NQ = TQ // 512       # 2 query free-dim tiles
EPS = 1e-5

_WNAMES = ["qt", "kf", "vf", "qf", "kt", "vt", "ot", "of"]


def _build_nc():
    import concourse.bass as bass
    import concourse.tile as tile
    from concourse import bacc, mybir
    from concourse.masks import make_identity
    from contextlib import ExitStack

    f32 = mybir.dt.float32
    bf16 = mybir.dt.bfloat16
    AF = mybir.ActivationFunctionType
    ALU = mybir.AluOpType

    nc = bacc.Bacc("TRN2", target_bir_lowering=False, debug=False, num_devices=8)

    # ---------------- DRAM I/O ----------------
    xtT_d = nc.dram_tensor("xtT", [D, T], bf16, kind="ExternalInput")
    xfT_d = nc.dram_tensor("xfT", [D, T], bf16, kind="ExternalInput")
    xtq_d = nc.dram_tensor("xtq", [TQ, D], f32, kind="ExternalInput")
    xfq_d = nc.dram_tensor("xfq", [TQ, D], f32, kind="ExternalInput")
    w_d = {}
    b_d = {}
    for n in _WNAMES:
        w_d[n] = nc.dram_tensor(f"w_{n}", [D, D], bf16, kind="ExternalInput")
        b_d[n] = nc.dram_tensor(f"b_{n}", [D], f32, kind="ExternalInput")
    w_d["f1"] = nc.dram_tensor("w_f1", [2 * D, D], bf16, kind="ExternalInput")
    b_d["f1"] = nc.dram_tensor("b_f1", [D], f32, kind="ExternalInput")
    w_d["f2"] = nc.dram_tensor("w_f2", [D, D], bf16, kind="ExternalInput")
    b_d["f2"] = nc.dram_tensor("b_f2", [D], f32, kind="ExternalInput")
    ln_d = {}
    for n in ["lnt_w", "lnt_b", "lnf_w", "lnf_b", "lnu_w", "lnu_b"]:
        ln_d[n] = nc.dram_tensor(n, [D], f32, kind="ExternalInput")
    out_d = nc.dram_tensor("out", [TQ, D], f32, kind="ExternalOutput")

    with tile.TileContext(nc) as tc, ExitStack() as ctx:
        const = ctx.enter_context(tc.tile_pool(name="const", bufs=1))
        wpool = ctx.enter_context(tc.tile_pool(name="wpool", bufs=2))
        res = ctx.enter_context(tc.tile_pool(name="res", bufs=1))
        xs = ctx.enter_context(tc.tile_pool(name="xs", bufs=2))
        kvs = ctx.enter_context(tc.tile_pool(name="kvs", bufs=2))
        ost = ctx.enter_context(tc.tile_pool(name="ost", bufs=2))
        stg = ctx.enter_context(tc.tile_pool(name="stg", bufs=3))
        ppool = ctx.enter_context(tc.tile_pool(name="ppool", bufs=3))
        spool = ctx.enter_context(tc.tile_pool(name="spool", bufs=2))
        lnp = ctx.enter_context(tc.tile_pool(name="lnp", bufs=2))
        rowp = ctx.enter_context(tc.tile_pool(name="rowp", bufs=1))
        dram = ctx.enter_context(tc.tile_pool(name="dram", bufs=1, space="DRAM"))
        ps_acc = ctx.enter_context(tc.tile_pool(name="ps_acc", bufs=2, space="PSUM"))
        ps_o = ctx.enter_context(tc.tile_pool(name="ps_o", bufs=3, space="PSUM"))
        ps_ln = ctx.enter_context(tc.tile_pool(name="ps_ln", bufs=1, space="PSUM"))

        ident = const.tile([P, P], bf16)
        make_identity(nc, ident[:])
        eps_t = const.tile([P, 1], f32, name="eps")
        nc.gpsimd.memset(eps_t[:], EPS)

        bias_col = {}
        for n in ["qt", "kf", "qf", "kt", "ot", "of", "f1", "f2"]:
            t = const.tile([P, DT], f32, name=f"bias_{n}")
            nc.sync.dma_start(t[:], b_d[n].rearrange("(dt p) -> p dt", p=P))
            bias_col[n] = t

        def row_bcast(dram_t, tag):
            """[D] f32 dram -> [128, D] bf16 broadcast tile."""
            r = rowp.tile([1, D], f32, tag="row")
            nc.sync.dma_start(r[:], dram_t.rearrange("(a d) -> a d", a=1))
            rb = rowp.tile([1, D], bf16, tag="rowb")
            nc.vector.tensor_copy(rb[:], r[:])
            b = rowp.tile([P, D], bf16, tag=tag)
            nc.gpsimd.partition_broadcast(b[:], rb[:])
            return b

        def load_weight(name, rows=None):
            dram_t = w_d[name]
            if rows is None:
                rows = (0, dram_t.shape[0])
            nkt = (rows[1] - rows[0]) // P
            t = wpool.tile([P, nkt, D], bf16, tag="w")
            nc.sync.dma_start(
                t[:],
                dram_t[rows[0]: rows[1], :].rearrange("(kt p) n -> p kt n", p=P),
            )
            return t

        # DRAM scratch
        kf_dr = dram.tile([D, T], bf16, name="kf_dr")
        kt_dr = dram.tile([D, T], bf16, name="kt_dr")
        vf_dr = dram.tile([T, NH, HD + 1], bf16, name="vf_dr")
        vt_dr = dram.tile([T, NH, HD + 1], bf16, name="vt_dr")
        ot_dr = dram.tile([D, TQ], bf16, name="ot_dr")
        of_dr = dram.tile([D, TQ], bf16, name="of_dr")

        # ones columns of V' (written once, before the V units)


        # ------------------------------------------------------------------
        # unit builders (each unit = one closure emitting one psum group)
        # ------------------------------------------------------------------
        def x_block_loader(x_dram, n0):
            blk = {}

            def get():
                if "xb" not in blk:
                    xb = xs.tile([P, DT, 512], bf16, tag="xs")
                    nc.sync.dma_start(
                        xb[:],
                        x_dram[:, n0: n0 + 512].rearrange(
                            "(dt p) t -> p dt t", p=P
                        ),
                    )
                    blk["xb"] = xb
                return blk["xb"]

            return get

        def featmaj_units(w_sb, bname, get_rhs, n0, sink, act=None):
            """y^T[dout, n0:n0+512] units; sink(dt, psum_ap) consumes."""
            units = []
            nkt = w_sb.shape[1]
            for dt in range(DT):

                def u(dt=dt):
                    ps = ps_acc.tile([P, 2, 512], f32, tag="acc")
                    rhs = get_rhs()
                    for kt in range(nkt):
                        nc.tensor.matmul(
                            ps[:, 0, :],
                            w_sb[:, kt, dt * P: (dt + 1) * P],
                            rhs[:, kt, :],
                            start=(kt == 0),
                            stop=(kt == nkt - 1),
                        )
                    sink(dt, ps[:, 0, :])

                units.append(u)
            return units

        def proj_to_dram_sink(bname, k_dr, n0, act=None):
            def sink(dt, ps):
                s = stg.tile([P, 512], bf16, tag="stg")
                nc.scalar.activation(
                    s[:], ps, AF.Identity, bias=bias_col[bname][:, dt: dt + 1]
                )
                nc.sync.dma_start(k_dr[dt * P: (dt + 1) * P, n0: n0 + 512], s[:])

            return sink

        def proj_to_sbuf_sink(bname, out_sb, n0, act=None):
            def sink(dt, ps):
                nc.scalar.activation(
                    out_sb[:, dt, n0: n0 + 512],
                    ps,
                    act if act is not None else AF.Identity,
                    bias=bias_col[bname][:, dt: dt + 1],
                )

            return sink

        def v_units(w_sb, vb_bc, get_x, n0, v_dr):
            """token-major V' units for token block n0 (4 chunks x 2 halves)."""
            units = []
            for tci in range(4):
                for no in range(2):

                    def u(tci=tci, no=no):
                        ps = ps_acc.tile([P, 2, 512], f32, tag="acc")
                        xb = get_x()
                        for kt in range(DT):
                            nc.tensor.matmul(
                                ps[:, 0, :],
                                xb[:, kt, tci * P: (tci + 1) * P],
                                w_sb[:, kt, no * 512: (no + 1) * 512],
                                start=(kt == 0),
                                stop=(kt == DT - 1),
                            )
                        s = stg.tile([P, 8, HD + 1], bf16, tag="stg")
                        nc.vector.tensor_add(
                            s[:, :, 0:HD],
                            ps[:, 0, :].rearrange("p (h e) -> p h e", h=8),
                            vb_bc[:, no * 512: (no + 1) * 512].rearrange(
                                "p (h e) -> p h e", h=8
                            ),
                        )
                        nc.gpsimd.memset(s[:, :, HD: HD + 1], 1.0)
                        tok0 = n0 + tci * P
                        nc.sync.dma_start(
                            v_dr[tok0: tok0 + P, no * 8: (no + 1) * 8, :],
                            s[:],
                        )

                    units.append(u)
            return units

        def attention_units(qT, k_dr, v_dr, o_dr):
            """One closure per (qt, head-pair). Streams K/V', writes O^T."""
            units = []
            for qt in range(NQ):
                for hp in range(NH // 2):

                    def u(qt=qt, hp=hp):
                        kS = kvs.tile([P, T], bf16, tag="kS")
                        nc.sync.dma_start(
                            kS[:], k_dr[hp * P: (hp + 1) * P, :]
                        )
                        vS = []
                        for sub in range(2):
                            v = kvs.tile([P, KT, HD + 1], bf16, tag="vS")
                            nc.sync.dma_start(
                                v[:],
                                v_dr[:, hp * 2 + sub, :].rearrange(
                                    "(kt p) e -> p kt e", p=P
                                ),
                            )
                            vS.append(v)
                        o_ps = [
                            ps_o.tile([P, 512], f32, tag="ops", name=f"o{s}")
                            for s in range(2)
                        ]
                        prev = None
                        for pr in range(KT // 2):
                            cur = []
                            for sub in range(2):
                                lo, hi = sub * HD, (sub + 1) * HD
                                s = ps_acc.tile([P, 2, 512], f32, tag="acc")
                                for j in range(2):
                                    kt = 2 * pr + j
                                    nc.tensor.matmul(
                                        s[:, j, :],
                                        kS[lo:hi, kt * P: (kt + 1) * P],
                                        qT[lo:hi, hp, qt * 512: (qt + 1) * 512],
                                        start=True,
                                        stop=True,
                                        tile_position=(lo, 0),
                                    )
                                pT = ppool.tile([P, 2, 512], bf16, tag="pT")
                                nc.scalar.activation(
                                    pT[:], s[:], AF.Exp, scale=1.0 / 8.0
                                )
                                cur.append((sub, pT))
                            # PV for previous pair (skewed to hide exp latency)
                            if prev is not None:
                                for sub, pTp in prev:
                                    for j in range(2):
                                        kt = 2 * (pr - 1) + j
                                        nc.tensor.matmul(
                                            o_ps[sub][0: HD + 1, :],
                                            vS[sub][:, kt, :],
                                            pTp[:, j, :],
                                            start=(kt == 0),
                                            stop=False,
                                        )
                            prev = cur
                        for sub, pTp in prev:
                            for j in range(2):
                                kt = KT - 2 + j
                                nc.tensor.matmul(
                                    o_ps[sub][0: HD + 1, :],
                                    vS[sub][:, kt, :],
                                    pTp[:, j, :],
                                    start=False,
                                    stop=(j == 1),
                                )
                        for sub in range(2):
                            inv = spool.tile([1, 512], f32, tag="inv")
                            nc.vector.reciprocal(inv[:], o_ps[sub][HD: HD + 1, :])
                            bc = spool.tile([HD, 512], f32, tag="bc")
                            nc.gpsimd.partition_broadcast(bc[:], inv[:])
                            s = stg.tile([HD, 512], bf16, tag="stg")
                            nc.vector.tensor_mul(s[:], o_ps[sub][0:HD, :], bc[:])
                            r0 = hp * P + sub * HD
                            nc.sync.dma_start(
                                o_dr[r0: r0 + HD, qt * 512: (qt + 1) * 512], s[:]
                            )

                    units.append(u)
            return units

        def oproj_units(w_sb, bname, o_dr, attnT):
            units = []
            for n0 in range(0, TQ, 512):
                get = {}

                def get_ob(n0=n0, get=get):
                    if "ob" not in get:
                        ob = ost.tile([P, DT, 512], bf16, tag="os")
                        nc.sync.dma_start(
                            ob[:],
                            o_dr[:, n0: n0 + 512].rearrange(
                                "(kt p) t -> p kt t", p=P
                            ),
                        )
                        get["ob"] = ob
                    return get["ob"]

                for dt in range(DT):

                    def u(dt=dt, n0=n0, get_ob=get_ob):
                        ps = ps_acc.tile([P, 2, 512], f32, tag="acc")
                        ob = get_ob()
                        for kt in range(DT):
                            nc.tensor.matmul(
                                ps[:, 0, :],
                                w_sb[:, kt, dt * P: (dt + 1) * P],
                                ob[:, kt, :],
                                start=(kt == 0),
                                stop=(kt == DT - 1),
                            )
                        nc.scalar.activation(
                            attnT[:, dt, n0: n0 + 512],
                            ps[:, 0, :],
                            AF.Identity,
                            bias=bias_col[bname][:, dt: dt + 1],
                        )

                    units.append(u)
            return units

        def ln_units(inT, resid_dram, w_bc, b_bc, outT, out_dram=None):
            """Token-major LN, one unit per 128-token chunk."""
            units = []
            for qc in range(QC):

                def u(qc=qc):
                    tok = ps_ln.tile([P, D], bf16, tag="lntok")
                    for dt in range(DT):
                        nc.tensor.transpose(
                            tok[:, dt * P: (dt + 1) * P],
                            inT[:, dt, qc * P: (qc + 1) * P],
                            ident[:],
                        )
                    s = lnp.tile([P, D], f32, tag="lnB")
                    if resid_dram is not None:
                        xq = lnp.tile([P, D], f32, tag="lnA")
                        nc.sync.dma_start(
                            xq[:], resid_dram[qc * P: (qc + 1) * P, :]
                        )
                        nc.vector.tensor_add(s[:], xq[:], tok[:])
                    else:
                        nc.vector.tensor_copy(s[:], tok[:])
                    bns = spool.tile([P, 2, 6], f32, tag="bns")
                    nc.vector.bn_stats(
                        bns[:], s.rearrange("p (a b) -> p a b", a=2)
                    )
                    mv = spool.tile([P, 2], f32, tag="mv")
                    nc.vector.bn_aggr(mv[:], bns[:])
                    std = spool.tile([P, 1], f32, tag="std")
                    nc.scalar.activation(std[:], mv[:, 1:2], AF.Sqrt, bias=eps_t[:])
                    rstd = spool.tile([P, 1], f32, tag="rstd")
                    nc.vector.reciprocal(rstd[:], std[:])
                    t1 = lnp.tile([P, D], f32, tag="lnA")
                    nc.vector.scalar_tensor_tensor(
                        t1[:], s[:], mv[:, 0:1], w_bc[:],
                        op0=ALU.subtract, op1=ALU.mult,
                    )
                    if out_dram is not None:
                        o = lnp.tile([P, D], f32, tag="lnB")
                        nc.vector.scalar_tensor_tensor(
                            o[:], t1[:], rstd[:], b_bc[:],
                            op0=ALU.mult, op1=ALU.add,
                        )
                        nc.sync.dma_start(out_dram[qc * P: (qc + 1) * P, :], o[:])
                    else:
                        nrm = lnp.tile([P, D], bf16, tag="lnnrm")
                        nc.vector.scalar_tensor_tensor(
                            nrm[:], t1[:], rstd[:], b_bc[:],
                            op0=ALU.mult, op1=ALU.add,
                        )
                        ft = ps_ln.tile([P, D], bf16, tag="lntok")
                        for dt in range(DT):
                            nc.tensor.transpose(
                                ft[:, dt * P: (dt + 1) * P],
                                nrm[:, dt * P: (dt + 1) * P],
                                ident[:],
                            )
                        nc.vector.tensor_copy(
                            outT[:, :, qc * P: (qc + 1) * P],
                            ft.rearrange("p (dt c) -> p dt c", dt=DT),
                        )

                units.append(u)
            return units

        def run_interleaved(primary, filler):
            k = 0
            for i, u in enumerate(primary):
                u()
                want = (i + 1) * len(filler) // len(primary)
                while k < want:
                    filler[k]()
                    k += 1
            while k < len(filler):
                filler[k]()
                k += 1

        # ------------------------------------------------------------------
        # program
        # ------------------------------------------------------------------
        # resident activation tiles (slot-shared by tag across phases)
        qT_t = res.tile([P, DT, TQ], bf16, name="qT_t", tag="qTt")
        qT_f = res.tile([P, DT, TQ], bf16, name="qT_f", tag="qTf")

        # Phase 1: Kf/Vf -> dram, Qt, Qf -> sbuf
        w_kf = load_weight("kf")
        w_vf = load_weight("vf")
        vb_f = row_bcast(b_d["vf"], "vbc")
        for n0 in range(0, T, 512):
            get_x = x_block_loader(xfT_d, n0)
            ku = featmaj_units(
                w_kf, "kf", get_x, n0, proj_to_dram_sink("kf", kf_dr, n0)
            )
            vu = v_units(w_vf, vb_f, get_x, n0, vf_dr)
            run_interleaved(ku, vu)
        w_qt = load_weight("qt")
        for n0 in range(0, TQ, 512):
            get_x = x_block_loader(xtT_d, n0)
            for u in featmaj_units(
                w_qt, "qt", get_x, n0, proj_to_sbuf_sink("qt", qT_t, n0)
            ):
                u()
        # Phase 2: attention-1 (streams kf/vf) || Kt/Vt/Qf projections
        w_kt = load_weight("kt")
        w_vt = load_weight("vt")
        w_qf = load_weight("qf")
        vb_t = row_bcast(b_d["vt"], "vbc")
        fillers = []
        for n0 in range(0, T, 512):
            get_x = x_block_loader(xtT_d, n0)
            fillers += featmaj_units(
                w_kt, "kt", get_x, n0, proj_to_dram_sink("kt", kt_dr, n0)
            )
            fillers += v_units(w_vt, vb_t, get_x, n0, vt_dr)
        for n0 in range(0, TQ, 512):
            get_x = x_block_loader(xfT_d, n0)
            fillers += featmaj_units(
                w_qf, "qf", get_x, n0, proj_to_sbuf_sink("qf", qT_f, n0)
            )
        run_interleaved(attention_units(qT_t, kf_dr, vf_dr, ot_dr), fillers)

        # Phase 3: attention-2 || O-proj(t) + LN(t)
        w_ot = load_weight("ot")
        attnT_t = res.tile([P, DT, TQ], bf16, name="attnT_t", tag="big")
        fusedT_t = res.tile([P, DT, TQ], bf16, name="fusedT_t", tag="qTt")
        lnt_wb = row_bcast(ln_d["lnt_w"], "lnw")
        lnt_bb = row_bcast(ln_d["lnt_b"], "lnb")
        fillers = oproj_units(w_ot, "ot", ot_dr, attnT_t)
        fillers += ln_units(attnT_t, xtq_d, lnt_wb, lnt_bb, fusedT_t)
        run_interleaved(attention_units(qT_f, kt_dr, vt_dr, of_dr), fillers)

        # Phases 4+5 (zippered): O-proj(f), LN(f), fus1, fus2, LN(fus) are a
        # pipeline over 512-token blocks; interleave so LN vector math hides
        # under the next stage's matmuls.
        w_of = load_weight("of")
        attnT_f = res.tile([P, DT, TQ], bf16, name="attnT_f", tag="big")
        fusedT_f = res.tile([P, DT, TQ], bf16, name="fusedT_f", tag="ff")
        lnf_wb = row_bcast(ln_d["lnf_w"], "lnw")
        lnf_bb = row_bcast(ln_d["lnf_b"], "lnb")
        oprojB = oproj_units(w_of, "of", of_dr, attnT_f)
        lnB = ln_units(attnT_f, xfq_d, lnf_wb, lnf_bb, fusedT_f)

        w_f1a = load_weight("f1", rows=(0, D))
        w_f1b = load_weight("f1", rows=(D, 2 * D))
        hT = res.tile([P, DT, TQ], bf16, name="hT", tag="qTf")

        def fus1_units(n0):
            units = []
            for dt in range(DT):

                def u(dt=dt, n0=n0):
                    ps = ps_acc.tile([P, 2, 512], f32, tag="acc")
                    for kt in range(DT):
                        nc.tensor.matmul(
                            ps[:, 0, :],
                            w_f1a[:, kt, dt * P: (dt + 1) * P],
                            fusedT_t[:, kt, n0: n0 + 512],
                            start=(kt == 0),
                            stop=False,
                        )
                    for kt in range(DT):
                        nc.tensor.matmul(
                            ps[:, 0, :],
                            w_f1b[:, kt, dt * P: (dt + 1) * P],
                            fusedT_f[:, kt, n0: n0 + 512],
                            start=False,
                            stop=(kt == DT - 1),
                        )
                    nc.scalar.activation(
                        hT[:, dt, n0: n0 + 512],
                        ps[:, 0, :],
                        AF.Gelu,
                        bias=bias_col["f1"][:, dt: dt + 1],
                    )

                units.append(u)
            return units

        w_f2 = load_weight("f2")
        o2T = res.tile([P, DT, TQ], bf16, name="o2T", tag="big")
        lnu_wb = row_bcast(ln_d["lnu_w"], "lnw")
        lnu_bb = row_bcast(ln_d["lnu_b"], "lnb")
        lnU = ln_units(o2T, None, lnu_wb, lnu_bb, None, out_dram=out_d)

        def fus2_units(n0):
            return featmaj_units(
                w_f2, "f2", lambda: hT[:, :, n0: n0 + 512], n0,
                proj_to_sbuf_sink("f2", o2T, n0),
            )

        for u in oprojB[:8]:
            u()
        run_interleaved(oprojB[8:], lnB[:4])
        run_interleaved(fus1_units(0), lnB[4:])
        for u in fus1_units(512):
            u()
        for u in fus2_units(0):
            u()
        run_interleaved(fus2_units(512), lnU[:4])
        for u in lnU[4:]:
            u()

    nc.compile()
    return nc


# ---------------------------------------------------------------------------
# host side
# ---------------------------------------------------------------------------
_CACHE = {}


def _get_nc():
    if "nc" not in _CACHE:
        _CACHE["nc"] = _build_nc()
    return _CACHE["nc"]


def _make_in_maps(inputs):
    import ml_dtypes

    bf16 = ml_dtypes.bfloat16
    t = np.asarray(inputs["temporal_tokens"], np.float32)
    f = np.asarray(inputs["feature_tokens"], np.float32)
    shared = {}
    for n in _WNAMES:
        shared[f"w_{n}"] = np.ascontiguousarray(inputs[f"{n}_w"]).astype(bf16)
        shared[f"b_{n}"] = np.ascontiguousarray(inputs[f"{n}_b"], dtype=np.float32)
    shared["w_f1"] = np.ascontiguousarray(inputs["fus1_w"]).astype(bf16)
    shared["b_f1"] = np.ascontiguousarray(inputs["fus1_b"], dtype=np.float32)
    shared["w_f2"] = np.ascontiguousarray(inputs["fus2_w"]).astype(bf16)
    shared["b_f2"] = np.ascontiguousarray(inputs["fus2_b"], dtype=np.float32)
    for src, dst in [
        ("ln_t_w", "lnt_w"), ("ln_t_b", "lnt_b"),
        ("ln_f_w", "lnf_w"), ("ln_f_b", "lnf_b"),
        ("ln_fus_w", "lnu_w"), ("ln_fus_b", "lnu_b"),
    ]:
        shared[dst] = np.ascontiguousarray(inputs[src], dtype=np.float32)

    in_maps = []
    for c in range(8):
        b, half = divmod(c, 2)
        r0 = half * TQ
        xt = t[b]
        xf = f[b]
        # query rows first, remaining rows after (K/V order is irrelevant)
        perm = np.concatenate([np.arange(r0, T), np.arange(0, r0)])
        m = dict(shared)
        m["xtT"] = np.ascontiguousarray(xt[perm].T).astype(bf16)
        m["xfT"] = np.ascontiguousarray(xf[perm].T).astype(bf16)
        m["xtq"] = np.ascontiguousarray(xt[r0: r0 + TQ])
        m["xfq"] = np.ascontiguousarray(xf[r0: r0 + TQ])
        in_maps.append(m)
    return in_maps


def kernel(**inputs):
    try:
        import jax

        jax.config.update("jax_compilation_cache_dir", "/tmp/jaxcache")
        jax.config.update("jax_persistent_cache_min_entry_size_bytes", -1)
        jax.config.update("jax_persistent_cache_min_compile_time_secs", 0.0)
    except Exception:
        pass
    from concourse.bass_utils import run_bass_kernel_spmd

    nc = _get_nc()
    in_maps = _make_in_maps(inputs)
    res = run_bass_kernel_spmd(nc, in_maps, list(range(8)))
    B = 4
    out = np.empty((B, T, D), np.float32)
    for c in range(8):
        b, half = divmod(c, 2)
        out[b, half * TQ: (half + 1) * TQ] = res.results[c]["out"]
    return out
